# revision 2
# baseline (speedup 1.0000x reference)
"""GQA attention kernel for Trainium2, sharded over 8 NeuronCores — v2.

Sharding: tensor-parallel over heads (as v1). Core c owns kv-head c and
q-heads 4c..4c+3; o_proj column-parallel; host sums the 8 bf16 partials.

v2 speedups over v1:
- All >=128-contraction matmuls (QKV, o_proj) run as fp8e4 DoubleRow with
  a hi+lo residual decomposition (3 terms, lo*lo dropped) -> 0.75 c/col
  per 128-k-tile instead of 1.0, at ~bf16 accuracy. Weights are prescaled
  by 32 on the host so their fp8 residuals stay above the e4m3 subnormal
  floor; the 1/32 descale folds into the PSUM drains.
- Scores (d=64 contraction) use a 4-slot exact trick: K's hi/lo stacked on
  the two partition halves (both DoubleRow j-slots), Q's hi/lo on the two
  j-slots (replicated across partition halves). One DR matmul computes the
  full (Khi+Klo)^T(Qhi+Qlo) product: 0.5 c/col, 2x over fp32r.
- exp outputs bf16; context matmul is exact bf16 (1.0 c/col).
- The softmax denominator ones-column is 1/32 so ctx comes out prescaled
  by 32 for its own fp8 hi/lo split feeding o_proj (descale 2^-10 in the
  output drain). Output partial is written bf16 (halves out DMA).
- Software pipelining: batch-1 QKV matmuls inject into batch-0's first
  attention block, o_proj of block k injects into block k+1, so ACT (exp)
  stays fed and PE never idles between phases.
"""

import os
import sys

for _p in ("/opt/trn_rl_repo",):
    if _p not in sys.path and os.path.isdir(_p):
        sys.path.insert(0, _p)

import numpy as np
import ml_dtypes

import concourse.bass as bass
import concourse.bacc as bacc
import concourse.tile as tile
from concourse import mybir
from concourse import bass_utils

F32 = mybir.dt.float32
F32R = mybir.dt.float32r
F8 = mybir.dt.float8e4
BF16 = mybir.dt.bfloat16
AF = mybir.ActivationFunctionType
ALU = mybir.AluOpType
DR = mybir.MatmulPerfMode.DoubleRow

FP8NP = ml_dtypes.float8_e4m3fn
BF16NP = ml_dtypes.bfloat16

B = 2
S = 2048
H = 2048
D = 64
N_CORES = 8
QF = 4 * D               # 256 q features per core
TOK = B * S              # 4096
CK = 512                 # tokens per QKV chunk
NCK = TOK // CK          # 8 chunks (4 per batch)
SCALE = 1.0 / np.sqrt(D)  # 0.125
WSC = 1.0 / 32.0         # weight descale after x32 host prescale

_CACHE = {}


def _build_program():
    nc = bacc.Bacc("TRN2", target_bir_lowering=False, debug=False)

    hsT_hi = nc.dram_tensor("hsT_hi", [H, TOK], F8, kind="ExternalInput").ap()
    hsT_lo = nc.dram_tensor("hsT_lo", [H, TOK], F8, kind="ExternalInput").ap()
    wqkvT_hi = nc.dram_tensor("wqkvT_hi", [128, 16, 384], F8, kind="ExternalInput").ap()
    wqkvT_lo = nc.dram_tensor("wqkvT_lo", [128, 16, 384], F8, kind="ExternalInput").ap()
    woT_hi = nc.dram_tensor("woT_hi", [128, 2, H], F8, kind="ExternalInput").ap()
    woT_lo = nc.dram_tensor("woT_lo", [128, 2, H], F8, kind="ExternalInput").ap()
    bqkv = nc.dram_tensor("bqkv", [128, 3], F32, kind="ExternalInput").ap()
    maskp = nc.dram_tensor("maskp", [128, B, S // 128], F32, kind="ExternalInput").ap()
    out = nc.dram_tensor("out", [B, S, H], BF16, kind="ExternalOutput").ap()

    hsT_hi_t = hsT_hi.rearrange("(t p) n -> p t n", p=128)
    hsT_lo_t = hsT_lo.rearrange("(t p) n -> p t n", p=128)

    with tile.TileContext(nc) as tc:
        with tc.tile_pool(name="const", bufs=1) as cp:
            bqkv_sb = cp.tile([128, 3], F32)
            nc.sync.dma_start(out=bqkv_sb, in_=bqkv)
            mask_sb = cp.tile([128, B, S // 128], F32)
            nc.sync.dma_start(out=mask_sb, in_=maskp)
            w_hi = cp.tile([128, 16, 384], F8)     # (p, h_tile, feature)
            nc.sync.dma_start(out=w_hi, in_=wqkvT_hi)
            w_lo = cp.tile([128, 16, 384], F8)
            wo_hi = cp.tile([128, 2, H], F8)       # (p, feat_half, e)
            wo_lo = cp.tile([128, 2, H], F8)
            # warm consumer engines' vector clocks on the small const DMAs
            scratch = cp.tile([128, 1], F32)
            nc.scalar.copy(out=scratch, in_=bqkv_sb[:, 0:1])
            nc.scalar.copy(out=scratch, in_=mask_sb[:, 0, 0:1])
            scratch2 = cp.tile([128, 1], F32)
            nc.vector.tensor_copy(out=scratch2, in_=bqkv_sb[:, 1:2])
            nc.gpsimd.tensor_copy(out=scratch2, in_=bqkv_sb[:, 2:3])

            # persistent activations, hi/lo interleaved on dim "hl"
            q_hl = cp.tile([128, 2, 2, TOK], F8)   # (p, feat_tile, hl, tok)
            k_hl = cp.tile([64, 2, TOK], F8)       # (p, hl, tok)
            vb = cp.tile([128, TOK], BF16)  # V lives in partitions 64:128
            # K stacked hi/lo on partition halves, replicated along j
            kstack = cp.tile([128, 2, TOK], F8)
            # Q hi/lo on j slots, replicated across partition halves; per (b,g)
            qrep = [[cp.tile([128, 2, S], F8, name=f"qrep_{b}_{g}")
                     for g in range(4)] for b in range(B)]
            # V[t,d] + 1/32 ones column, per 128-token tile
            vones = cp.tile([128, B * 16, 65], BF16)
            nc.vector.memset(vones[:, :, 64:65], 1.0 / 32.0)

            # ctx (x32, normalized) hi/lo, stacked [feat_half j] for o_proj
            # one pair per attention block, double buffered
            ctx_hi = [cp.tile([128, 2, 1024], F8, name=f"ctx_hi{i}") for i in range(2)]
            ctx_lo = [cp.tile([128, 2, 1024], F8, name=f"ctx_lo{i}") for i in range(2)]

            with tc.tile_pool(name="drain_sb", bufs=3) as dsb, \
                 tc.tile_pool(name="att_sb", bufs=3) as asb, \
                 tc.tile_pool(name="scores_ps", bufs=2, space="PSUM") as sps, \
                 tc.tile_pool(name="ctx_ps", bufs=2, space="PSUM") as xps:

                pools = {}

                def dummy_mm(tgt, src):
                    # spend one sync-wait on the PE clock: tiny matmul that
                    # reads 4 bytes of `src` and scribbles on a PSUM corner
                    # that a later start=True accumulation will reset.
                    nc.tensor.matmul(tgt, src, src, start=True, stop=True,
                                     skip_group_check=True)

                # ------------- QKV projection, fine-grained steps ----------
                # returns a list of ~1us closures for injection scheduling
                def qkv_chunk_steps(ck):
                    state = {}

                    def dma_step():
                        psb = pools["psb"]
                        hst_hi = psb.tile([128, 16, CK], F8, tag="hst_hi",
                                          name=f"hst_hi_{ck}")
                        hst_lo = psb.tile([128, 16, CK], F8, tag="hst_lo",
                                          name=f"hst_lo_{ck}")
                        nc.sync.dma_start(out=hst_hi,
                                          in_=hsT_hi_t[:, :, ck * CK:(ck + 1) * CK])
                        nc.sync.dma_start(out=hst_lo,
                                          in_=hsT_lo_t[:, :, ck * CK:(ck + 1) * CK])
                        state["hst"] = (hst_hi, hst_lo)

                    def mm_step(ft, qc):
                        # one full accumulation group per 256-col PSUM region:
                        # groups must NOT interleave on HW (a later start=True
                        # loses the other region's in-flight accumulation).
                        pps = pools["pps"]
                        hst_hi, hst_lo = state["hst"]
                        terms = ((w_hi, hst_hi), (w_hi, hst_lo), (w_lo, hst_hi))
                        if qc == 0:
                            state[ft] = pps.tile([128, CK], F32, tag="projps",
                                                 bufs=2, name=f"projps_{ck}_{ft}")
                            if ft == 2:
                                dummy_mm(state[ft][0:1, 0:1],
                                         hst_hi[0:1, 0, 0:4].bitcast(F32))
                                dummy_mm(state[ft][0:1, 0:1],
                                         hst_lo[0:1, 0, 0:4].bitcast(F32))
                        ps = state[ft]
                        for term in range(3):
                            wt, ht = terms[term]
                            for p in range(8):
                                nc.tensor.matmul(
                                    ps[:, qc * 256:(qc + 1) * 256],
                                    wt[:, 2 * p:2 * p + 2, ft * 128:(ft + 1) * 128],
                                    ht[:, 2 * p:2 * p + 2, qc * 256:(qc + 1) * 256],
                                    start=(term == 0 and p == 0),
                                    stop=(term == 2 and p == 7),
                                    perf_mode=DR,
                                )
                        if qc < CK // 256 - 1:
                            return
                        # drains: hi = fp8((ps + b*32) * 2^-5) in one pass;
                        # lo = ps*2^-5 - hi (bias enters at hi precision).
                        cs = ck * CK
                        if ft < 2:
                            hi = q_hl[:, ft, 0, cs:cs + CK]
                            nc.vector.tensor_scalar(
                                out=hi, in0=ps, scalar1=bqkv_sb[:, ft:ft + 1],
                                scalar2=WSC, op0=ALU.add, op1=ALU.mult)
                            nc.vector.scalar_tensor_tensor(
                                out=q_hl[:, ft, 1, cs:cs + CK], in0=ps, scalar=WSC,
                                in1=hi, op0=ALU.mult, op1=ALU.subtract)
                        else:
                            khi = k_hl[:, 0, cs:cs + CK]
                            nc.vector.tensor_scalar(
                                out=khi, in0=ps[0:64, :], scalar1=bqkv_sb[0:64, 2:3],
                                scalar2=WSC, op0=ALU.add, op1=ALU.mult)
                            nc.vector.scalar_tensor_tensor(
                                out=k_hl[:, 1, cs:cs + CK], in0=ps[0:64, :],
                                scalar=WSC, in1=khi, op0=ALU.mult, op1=ALU.subtract)
                            nc.vector.tensor_scalar(
                                out=vb[64:128, cs:cs + CK], in0=ps[64:128, :],
                                scalar1=bqkv_sb[64:128, 2:3],
                                scalar2=WSC, op0=ALU.add, op1=ALU.mult)

                    steps = [dma_step]
                    for ft in (2, 0, 1):
                        for qc in range(CK // 256):
                            steps.append(lambda ft=ft, qc=qc: mm_step(ft, qc))
                    return steps

                def qkv_chunk(ck):
                    for s in qkv_chunk_steps(ck):
                        s()

                # ------------- post-QKV rep builds for one batch -----------
                def rep_build(b, hb):
                    # hb: token half of the batch (granularity lets the
                    # scheduler start attention before the whole batch's QKV)
                    HS = S // 2
                    t0 = b * S + hb * HS
                    sl = np.s_[t0:t0 + HS]
                    for j in range(2):
                        nc.sync.dma_start(out=kstack[0:64, j, sl],
                                          in_=k_hl[:, 0, sl])
                        nc.sync.dma_start(out=kstack[64:128, j, sl],
                                          in_=k_hl[:, 1, sl])
                    def qrep_dma(g):
                        qp = (g % 2) * 64
                        ft = g // 2
                        for half in range(2):
                            nc.sync.dma_start(
                                out=qrep[b][g][64 * half:64 * half + 64, :,
                                               hb * HS:(hb + 1) * HS],
                                in_=q_hl[qp:qp + 64, ft, :, sl])
                    qrep_dma(0)
                    # V transpose for this half: xbar into a contiguous
                    # staging tile (the xbar cannot write gapped APs), then
                    # copy into vones' 65-stride layout.
                    vtr = dsb.tile([128, 8, 64], BF16, tag="vtr",
                                   name=f"vtr_{b}_{hb}")
                    nc.sync.dma_start_transpose(out=vtr, in_=vb[64:128, sl])
                    nc.gpsimd.tensor_copy(
                        out=vones[:, b * 16 + hb * 8:b * 16 + (hb + 1) * 8, 0:64],
                        in_=vtr)
                    for g in (1, 2, 3):
                        qrep_dma(g)

                # ---------------- o_proj for one block ---------------------
                # emits a list of closures, executed interleaved later
                def oproj_steps(blk, ops_pool, osb_pool, tail=False):
                    b, qh = blk // 2, blk % 2
                    chi, clo = ctx_hi[blk % 2], ctx_lo[blk % 2]
                    ec_per = 4 if tail else 2     # ec chunks per PSUM tile
                    steps = []
                    for tt in range(8):
                        def mk_tile(tt=tt):
                            osb = osb_pool.tile([128, H], BF16, tag="osb",
                                                name=f"osb_{blk}_{tt}")
                            nc.vector.memset(osb[0:1, 0:1], 0.0)
                            return osb
                        holder = {}
                        for ec in range(8):
                            def mm(tt=tt, ec=ec, holder=holder, mk_tile=mk_tile):
                                if "osb" not in holder:
                                    holder["osb"] = mk_tile()
                                if ec % ec_per == 0:
                                    if tail:
                                        holder["op"] = sps.tile(
                                            [128, 1024], F32, tag="scores",
                                            name=f"opst_{blk}_{tt}_{ec}")
                                    else:
                                        holder["op"] = ops_pool.tile(
                                            [128, 512], F32, tag="ops",
                                            name=f"ops_{blk}_{tt}_{ec}")
                                op = holder["op"]
                                oc = (ec % ec_per) * 256
                                first = True
                                for ct, wt in ((chi, wo_hi), (chi, wo_lo), (clo, wo_hi)):
                                    nc.tensor.matmul(
                                        op[:, oc:oc + 256],
                                        ct[:, :, tt * 128:(tt + 1) * 128],
                                        wt[:, :, ec * 256:(ec + 1) * 256],
                                        start=first, stop=(ct is clo),
                                        perf_mode=DR,
                                    )
                                    first = False
                                if ec % 2 == 1:
                                    osb = holder["osb"]
                                    di = tt * 4 + ec // 2
                                    if tail and di % 2 == 1:
                                        nc.scalar.activation(
                                            out=osb[:, (ec - 1) * 256:(ec + 1) * 256],
                                            in_=op[:, (ec % ec_per - 1) * 256:
                                                   (ec % ec_per + 1) * 256],
                                            func=AF.Copy, scale=1.0 / 1024.0)
                                    else:
                                        nc.vector.tensor_scalar(
                                            out=osb[:, (ec - 1) * 256:(ec + 1) * 256],
                                            in0=op[:, (ec % ec_per - 1) * 256:
                                                   (ec % ec_per + 1) * 256],
                                            scalar1=1.0 / 1024.0, scalar2=None,
                                            op0=ALU.mult)
                                if ec == 7:
                                    nc.sync.dma_start(
                                        out=out[b, qh * 1024 + tt * 128:
                                                qh * 1024 + (tt + 1) * 128, :],
                                        in_=holder["osb"])
                            steps.append(mm)
                    return steps

                # ---------------- attention block --------------------------
                def attention_block(blk, inject):
                    """inject: list of closures to interleave into the t-loop."""
                    b, qh = blk // 2, blk % 2
                    q0 = qh * 1024
                    chi, clo = ctx_hi[blk % 2], ctx_lo[blk % 2]
                    nc.vector.memset(chi[0:1, 0, 0:4].bitcast(F32), 0.0)
                    nc.vector.memset(clo[0:1, 0, 0:4].bitcast(F32), 0.0)
                    ninj = len(inject)
                    inj_i = 0
                    slots = 4 * 16  # g x t iterations
                    # software pipeline: ctx matmuls run one t behind scores,
                    # so PE never waits on the exp of the current tile.
                    pending = None  # (ctxs, ex, t)

                    def do_ctx(ctxs, ex, t):
                        if t == 0:
                            for c in ctxs:
                                nc.tensor.matmul(
                                    c[0:1, 0:1], w_hi[:, 0, 0:4].bitcast(F32),
                                    w_hi[:, 0, 0:4].bitcast(F32), start=True,
                                    stop=True, skip_group_check=True)
                        for qc2 in range(2):
                            nc.tensor.matmul(
                                ctxs[qc2],
                                vones[:, b * 16 + t, :],
                                ex[:, qc2 * 512:(qc2 + 1) * 512],
                                start=(t == 0), stop=(t == 15),
                            )

                    def drain_g(g, ctxs):
                        qp = (g % 2) * 64
                        for qc2 in range(2):
                            # fast PSUM release: copy to SBUF staging first
                            st = dsb.tile([65, 512], F32, tag="cstage",
                                          name=f"st_{blk}_{g}_{qc2}")
                            nc.vector.tensor_copy(out=st, in_=ctxs[qc2])
                            rc = dsb.tile([1, 512], F32, tag="recip",
                                          name=f"rc_{blk}_{g}_{qc2}")
                            nc.vector.reciprocal(out=rc, in_=st[64:65, :])
                            bc = dsb.tile([64, 512], F32, tag="bcast",
                                          name=f"bc_{blk}_{g}_{qc2}")
                            nc.gpsimd.partition_broadcast(bc, rc)
                            mc = dsb.tile([128, 512], F32, tag="cm",
                                          name=f"mc_{blk}_{g}_{qc2}")
                            mch = mc[qp:qp + 64, :]
                            nc.vector.tensor_mul(out=mch, in0=st[0:64, :], in1=bc)
                            dst = np.s_[qp:qp + 64, g // 2, qc2 * 512:(qc2 + 1) * 512]
                            nc.gpsimd.tensor_copy(out=chi[dst], in_=mch)
                            nc.gpsimd.tensor_sub(out=clo[dst], in0=mch, in1=chi[dst])

                    for g in range(4):
                        ctx0 = xps.tile([65, 512], F32, tag="ctx",
                                        name=f"ctx0_{blk}_{g}")
                        ctx1 = xps.tile([65, 512], F32, tag="ctx",
                                        name=f"ctx1_{blk}_{g}")
                        ctxs = (ctx0, ctx1)
                        for t in range(16):
                            sc = sps.tile([128, 1024], F32, tag="scores",
                                          name=f"sc_{blk}_{g}_{t}")
                            tok = b * S + t * 128
                            for qc in range(4):
                                nc.tensor.matmul(
                                    sc[:, qc * 256:(qc + 1) * 256],
                                    kstack[:, :, tok:tok + 128],
                                    qrep[b][g][:, :, q0 + qc * 256:q0 + (qc + 1) * 256],
                                    start=True, stop=True, perf_mode=DR,
                                )
                            ex = asb.tile([128, 1024], BF16, tag="expT",
                                          name=f"ex_{blk}_{g}_{t}")
                            nc.scalar.activation(
                                out=ex, in_=sc, func=AF.Exp,
                                bias=mask_sb[:, b, t:t + 1], scale=SCALE,
                            )
                            if pending is not None:
                                pctxs, pex, pt = pending
                                do_ctx(pctxs, pex, pt)
                                if pt == 15:
                                    drain_g(g - 1, pctxs)
                            pending = (ctxs, ex, t)
                            # interleave foreign PE work (avoid g boundaries)
                            if 1 < t < 14:
                                it = g * 12 + (t - 2)
                                want = ((it + 1) * ninj) // (4 * 12)
                                while inj_i < want:
                                    inject[inj_i]()
                                    inj_i += 1
                    pctxs, pex, pt = pending
                    do_ctx(pctxs, pex, pt)
                    drain_g(3, pctxs)
                    while inj_i < ninj:
                        inject[inj_i]()
                        inj_i += 1

                # ================= schedule ================================
                with tc.tile_pool(name="proj_sb", bufs=2) as psb, \
                     tc.tile_pool(name="proj_ps", bufs=2, space="PSUM") as pps:
                    pools["psb"], pools["pps"] = psb, pps
                    ck0_steps = qkv_chunk_steps(0)
                    ck0_steps[0]()               # chunk-0 hst DMAs first
                    nc.sync.dma_start(out=w_lo, in_=wqkvT_lo)
                    for s in ck0_steps[1:]:
                        s()
                    qkv_chunk(1)
                    rep_build(0, 0)
                    for ck in range(2, 4):       # batch-0 second half
                        qkv_chunk(ck)
                    rep_build(0, 1)
                    # batch-1 QKV injects into block 0
                    b1_steps = []
                    b1_steps.extend(qkv_chunk_steps(4))
                    b1_steps.extend(qkv_chunk_steps(5))
                    b1_steps.append(lambda: rep_build(1, 0))
                    b1_steps.extend(qkv_chunk_steps(6))
                    b1_steps.extend(qkv_chunk_steps(7))
                    b1_steps.append(lambda: rep_build(1, 1))
                    b1_steps.append(lambda: (
                        nc.sync.dma_start(out=wo_hi, in_=woT_hi),
                        nc.sync.dma_start(out=wo_lo, in_=woT_lo)))
                    attention_block(0, b1_steps)

                # o_proj(k) injects into block k+1
                with tc.tile_pool(name="o_ps", bufs=2, space="PSUM") as ops_pool, \
                     tc.tile_pool(name="osb_sb", bufs=3) as osb_pool:
                    attention_block(1, oproj_steps(0, ops_pool, osb_pool))
                    attention_block(2, oproj_steps(1, ops_pool, osb_pool))
                    attention_block(3, oproj_steps(2, ops_pool, osb_pool))
                    for step in oproj_steps(3, ops_pool, osb_pool, tail=True):
                        step()
    nc.compile()
    return nc


def _hi_lo(x):
    hi = np.asarray(x, FP8NP)
    lo = np.asarray(x - hi.astype(np.float32), FP8NP)
    return hi, lo


def kernel(hidden_states, attention_mask, Wq, bq, Wk, bk, Wv, bv, Wo, bo):
    hidden_states = np.asarray(hidden_states, dtype=np.float32)
    attention_mask = np.asarray(attention_mask, dtype=np.float32)
    Wq = np.asarray(Wq, dtype=np.float32)
    Wk = np.asarray(Wk, dtype=np.float32)
    Wv = np.asarray(Wv, dtype=np.float32)
    Wo = np.asarray(Wo, dtype=np.float32)

    if "nc" not in _CACHE:
        _CACHE["nc"] = _build_program()
    nc = _CACHE["nc"]

    hsT = np.ascontiguousarray(hidden_states.reshape(TOK, H).T)   # [H, TOK]
    hsT_hi, hsT_lo = _hi_lo(hsT)
    maskp = np.ascontiguousarray(
        attention_mask.reshape(B, S // 128, 128).transpose(2, 0, 1))  # [128, B, 16]
    in_maps = []
    for c in range(N_CORES):
        w = np.concatenate([Wq[QF * c:QF * (c + 1)],
                            Wk[D * c:D * (c + 1)],
                            Wv[D * c:D * (c + 1)]], axis=0) * 32.0   # [384, H]
        # pretile to SBUF layout [128 p, 16 htile, 384 f]: h = t*128 + p
        wqkvT = np.ascontiguousarray(
            w.T.reshape(16, 128, 384).transpose(1, 0, 2))
        wq_hi, wq_lo = _hi_lo(wqkvT)
        # pretile Wo^T slice to [128 p, 2 j, H]: ctx feat = j*128 + p
        woT = np.ascontiguousarray(
            (Wo[:, QF * c:QF * (c + 1)].T * 32.0).reshape(2, 128, H)
            .transpose(1, 0, 2))
        wo_hi, wo_lo = _hi_lo(woT)
        bqkv = np.ascontiguousarray(
            np.concatenate([bq[QF * c:QF * (c + 1)], bk[D * c:D * (c + 1)],
                            bv[D * c:D * (c + 1)]]).astype(np.float32)
            .reshape(3, 128).T) * 32.0                               # [128, 3]
        in_maps.append({
            "hsT_hi": hsT_hi, "hsT_lo": hsT_lo,
            "wqkvT_hi": wq_hi, "wqkvT_lo": wq_lo,
            "woT_hi": wo_hi, "woT_lo": wo_lo,
            "bqkv": bqkv, "maskp": maskp,
        })

    _CACHE["last_in_maps"] = in_maps
    res = bass_utils.run_bass_kernel_spmd(nc, in_maps, core_ids=list(range(N_CORES)))
    acc = np.zeros((B, S, H), dtype=np.float32)
    for c in range(N_CORES):
        acc += res.results[c]["out"].astype(np.float32)
    acc += np.asarray(bo, dtype=np.float32)[None, None, :]
    return acc


# revision 3
# speedup vs baseline: 1.0025x; 1.0025x over previous
"""GQA attention kernel for Trainium2, sharded over 8 NeuronCores — v2.

Sharding: tensor-parallel over heads (as v1). Core c owns kv-head c and
q-heads 4c..4c+3; o_proj column-parallel; host sums the 8 bf16 partials.

v2 speedups over v1:
- All >=128-contraction matmuls (QKV, o_proj) run as fp8e4 DoubleRow with
  a hi+lo residual decomposition (3 terms, lo*lo dropped) -> 0.75 c/col
  per 128-k-tile instead of 1.0, at ~bf16 accuracy. Weights are prescaled
  by 32 on the host so their fp8 residuals stay above the e4m3 subnormal
  floor; the 1/32 descale folds into the PSUM drains.
- Scores (d=64 contraction) use a 4-slot exact trick: K's hi/lo stacked on
  the two partition halves (both DoubleRow j-slots), Q's hi/lo on the two
  j-slots (replicated across partition halves). One DR matmul computes the
  full (Khi+Klo)^T(Qhi+Qlo) product: 0.5 c/col, 2x over fp32r.
- exp outputs bf16; context matmul is exact bf16 (1.0 c/col).
- The softmax denominator ones-column is 1/32 so ctx comes out prescaled
  by 32 for its own fp8 hi/lo split feeding o_proj (descale 2^-10 in the
  output drain). Output partial is written bf16 (halves out DMA).
- Software pipelining: batch-1 QKV matmuls inject into batch-0's first
  attention block, o_proj of block k injects into block k+1, so ACT (exp)
  stays fed and PE never idles between phases.
"""

import os
import sys

for _p in ("/opt/trn_rl_repo",):
    if _p not in sys.path and os.path.isdir(_p):
        sys.path.insert(0, _p)

import numpy as np
import ml_dtypes

import concourse.bass as bass
import concourse.bacc as bacc
import concourse.tile as tile
from concourse import mybir
from concourse import bass_utils

F32 = mybir.dt.float32
F32R = mybir.dt.float32r
F8 = mybir.dt.float8e4
BF16 = mybir.dt.bfloat16
AF = mybir.ActivationFunctionType
ALU = mybir.AluOpType
DR = mybir.MatmulPerfMode.DoubleRow

FP8NP = ml_dtypes.float8_e4m3fn
BF16NP = ml_dtypes.bfloat16

B = 2
S = 2048
H = 2048
D = 64
N_CORES = 8
QF = 4 * D               # 256 q features per core
TOK = B * S              # 4096
CK = 512                 # tokens per QKV chunk
NCK = TOK // CK          # 8 chunks (4 per batch)
SCALE = 1.0 / np.sqrt(D)  # 0.125
WSC = 1.0 / 32.0         # weight descale after x32 host prescale

_CACHE = {}


def _build_program():
    nc = bacc.Bacc("TRN2", target_bir_lowering=False, debug=False)

    hsT_hi = nc.dram_tensor("hsT_hi", [H, TOK], F8, kind="ExternalInput").ap()
    hsT_lo = nc.dram_tensor("hsT_lo", [H, TOK], F8, kind="ExternalInput").ap()
    wqkvT_hi = nc.dram_tensor("wqkvT_hi", [128, 16, 384], F8, kind="ExternalInput").ap()
    wqkvT_lo = nc.dram_tensor("wqkvT_lo", [128, 16, 384], F8, kind="ExternalInput").ap()
    woT_hi = nc.dram_tensor("woT_hi", [128, 2, H], F8, kind="ExternalInput").ap()
    woT_lo = nc.dram_tensor("woT_lo", [128, 2, H], F8, kind="ExternalInput").ap()
    bqkv = nc.dram_tensor("bqkv", [128, 3], F32, kind="ExternalInput").ap()
    maskp = nc.dram_tensor("maskp", [128, B, S // 128], F32, kind="ExternalInput").ap()
    out = nc.dram_tensor("out", [B, S, H], BF16, kind="ExternalOutput").ap()

    hsT_hi_t = hsT_hi.rearrange("(t p) n -> p t n", p=128)
    hsT_lo_t = hsT_lo.rearrange("(t p) n -> p t n", p=128)

    with tile.TileContext(nc) as tc:
        with tc.tile_pool(name="const", bufs=1) as cp:
            bqkv_sb = cp.tile([128, 3], F32)
            nc.sync.dma_start(out=bqkv_sb, in_=bqkv)
            mask_sb = cp.tile([128, B, S // 128], F32)
            nc.sync.dma_start(out=mask_sb, in_=maskp)
            w_hi = cp.tile([128, 16, 384], F8)     # (p, h_tile, feature)
            nc.sync.dma_start(out=w_hi, in_=wqkvT_hi)
            w_lo = cp.tile([128, 16, 384], F8)
            wo_hi = cp.tile([128, 2, H], F8)       # (p, feat_half, e)
            wo_lo = cp.tile([128, 2, H], F8)
            # warm consumer engines' vector clocks on the small const DMAs
            scratch = cp.tile([128, 1], F32)
            nc.scalar.copy(out=scratch, in_=bqkv_sb[:, 0:1])
            nc.scalar.copy(out=scratch, in_=mask_sb[:, 0, 0:1])
            scratch2 = cp.tile([128, 1], F32)
            nc.vector.tensor_copy(out=scratch2, in_=bqkv_sb[:, 1:2])
            nc.gpsimd.tensor_copy(out=scratch2, in_=bqkv_sb[:, 2:3])

            # persistent activations, hi/lo interleaved on dim "hl"
            q_hl = cp.tile([128, 2, 2, TOK], F8)   # (p, feat_tile, hl, tok)
            k_hl = cp.tile([64, 2, TOK], F8)       # (p, hl, tok)
            vb = cp.tile([128, TOK], BF16)  # V lives in partitions 64:128
            # K stacked hi/lo on partition halves, replicated along j
            kstack = cp.tile([128, 2, TOK], F8)
            # Q hi/lo on j slots, replicated across partition halves; per (b,g)
            qrep = [[cp.tile([128, 2, S], F8, name=f"qrep_{b}_{g}")
                     for g in range(4)] for b in range(B)]
            # V[t,d] + 1/32 ones column, per 128-token tile
            vones = cp.tile([128, B * 16, 65], BF16)
            nc.vector.memset(vones[:, :, 64:65], 1.0 / 32.0)

            # ctx (x32, normalized) hi/lo, stacked [feat_half j] for o_proj
            # one pair per attention block, double buffered
            ctx_hi = [cp.tile([128, 2, 1024], F8, name=f"ctx_hi{i}") for i in range(2)]
            ctx_lo = [cp.tile([128, 2, 1024], F8, name=f"ctx_lo{i}") for i in range(2)]

            with tc.tile_pool(name="drain_sb", bufs=3) as dsb, \
                 tc.tile_pool(name="att_sb", bufs=3) as asb, \
                 tc.tile_pool(name="scores_ps", bufs=2, space="PSUM") as sps, \
                 tc.tile_pool(name="ctx_ps", bufs=2, space="PSUM") as xps:

                pools = {}

                def dummy_mm(tgt, src):
                    # spend one sync-wait on the PE clock: tiny matmul that
                    # reads 4 bytes of `src` and scribbles on a PSUM corner
                    # that a later start=True accumulation will reset.
                    nc.tensor.matmul(tgt, src, src, start=True, stop=True,
                                     skip_group_check=True)

                # ------------- QKV projection, fine-grained steps ----------
                # returns a list of ~1us closures for injection scheduling
                def qkv_chunk_steps(ck):
                    state = {}

                    def dma_step():
                        psb = pools["psb"]
                        hst_hi = psb.tile([128, 16, CK], F8, tag="hst_hi",
                                          name=f"hst_hi_{ck}")
                        hst_lo = psb.tile([128, 16, CK], F8, tag="hst_lo",
                                          name=f"hst_lo_{ck}")
                        nc.sync.dma_start(out=hst_hi,
                                          in_=hsT_hi_t[:, :, ck * CK:(ck + 1) * CK])
                        nc.sync.dma_start(out=hst_lo,
                                          in_=hsT_lo_t[:, :, ck * CK:(ck + 1) * CK])
                        state["hst"] = (hst_hi, hst_lo)

                    def mm_step(ft, qc):
                        # one full accumulation group per 256-col PSUM region:
                        # groups must NOT interleave on HW (a later start=True
                        # loses the other region's in-flight accumulation).
                        pps = pools["pps"]
                        hst_hi, hst_lo = state["hst"]
                        terms = ((w_hi, hst_hi), (w_hi, hst_lo), (w_lo, hst_hi))
                        if qc == 0:
                            state[ft] = pps.tile([128, CK], F32, tag="projps",
                                                 bufs=2, name=f"projps_{ck}_{ft}")
                            if ft == 2:
                                dummy_mm(state[ft][0:1, 0:1],
                                         hst_hi[0:1, 0, 0:4].bitcast(F32))
                                dummy_mm(state[ft][0:1, 0:1],
                                         hst_lo[0:1, 0, 0:4].bitcast(F32))
                        ps = state[ft]
                        for term in range(3):
                            wt, ht = terms[term]
                            for p in range(8):
                                nc.tensor.matmul(
                                    ps[:, qc * 256:(qc + 1) * 256],
                                    wt[:, 2 * p:2 * p + 2, ft * 128:(ft + 1) * 128],
                                    ht[:, 2 * p:2 * p + 2, qc * 256:(qc + 1) * 256],
                                    start=(term == 0 and p == 0),
                                    stop=(term == 2 and p == 7),
                                    perf_mode=DR,
                                )
                        if qc < CK // 256 - 1:
                            return
                        # drains: hi = fp8((ps + b*32) * 2^-5) in one pass;
                        # lo = ps*2^-5 - hi (bias enters at hi precision).
                        cs = ck * CK
                        if ft < 2:
                            hi = q_hl[:, ft, 0, cs:cs + CK]
                            nc.vector.tensor_scalar(
                                out=hi, in0=ps, scalar1=bqkv_sb[:, ft:ft + 1],
                                scalar2=WSC, op0=ALU.add, op1=ALU.mult)
                            nc.vector.scalar_tensor_tensor(
                                out=q_hl[:, ft, 1, cs:cs + CK], in0=ps, scalar=WSC,
                                in1=hi, op0=ALU.mult, op1=ALU.subtract)
                        else:
                            khi = k_hl[:, 0, cs:cs + CK]
                            nc.vector.tensor_scalar(
                                out=khi, in0=ps[0:64, :], scalar1=bqkv_sb[0:64, 2:3],
                                scalar2=WSC, op0=ALU.add, op1=ALU.mult)
                            nc.vector.scalar_tensor_tensor(
                                out=k_hl[:, 1, cs:cs + CK], in0=ps[0:64, :],
                                scalar=WSC, in1=khi, op0=ALU.mult, op1=ALU.subtract)
                            nc.vector.tensor_scalar(
                                out=vb[64:128, cs:cs + CK], in0=ps[64:128, :],
                                scalar1=bqkv_sb[64:128, 2:3],
                                scalar2=WSC, op0=ALU.add, op1=ALU.mult)

                    steps = [dma_step]
                    for ft in (2, 0, 1):
                        for qc in range(CK // 256):
                            steps.append(lambda ft=ft, qc=qc: mm_step(ft, qc))
                    return steps

                def qkv_chunk(ck):
                    for s in qkv_chunk_steps(ck):
                        s()

                # ------------- post-QKV rep builds for one batch -----------
                def rep_build(b, hb):
                    # hb: token half of the batch (granularity lets the
                    # scheduler start attention before the whole batch's QKV)
                    HS = S // 2
                    t0 = b * S + hb * HS
                    sl = np.s_[t0:t0 + HS]
                    for j in range(2):
                        nc.sync.dma_start(out=kstack[0:64, j, sl],
                                          in_=k_hl[:, 0, sl])
                        nc.sync.dma_start(out=kstack[64:128, j, sl],
                                          in_=k_hl[:, 1, sl])
                    def qrep_dma(g):
                        qp = (g % 2) * 64
                        ft = g // 2
                        for half in range(2):
                            nc.sync.dma_start(
                                out=qrep[b][g][64 * half:64 * half + 64, :,
                                               hb * HS:(hb + 1) * HS],
                                in_=q_hl[qp:qp + 64, ft, :, sl])
                    qrep_dma(0)
                    # V transpose for this half: xbar into a contiguous
                    # staging tile (the xbar cannot write gapped APs), then
                    # copy into vones' 65-stride layout.
                    vtr = dsb.tile([128, 8, 64], BF16, tag="vtr",
                                   name=f"vtr_{b}_{hb}")
                    nc.sync.dma_start_transpose(out=vtr, in_=vb[64:128, sl])
                    nc.gpsimd.tensor_copy(
                        out=vones[:, b * 16 + hb * 8:b * 16 + (hb + 1) * 8, 0:64],
                        in_=vtr)
                    for g in (1, 2, 3):
                        qrep_dma(g)

                # ---------------- o_proj for one block ---------------------
                # emits a list of closures, executed interleaved later
                def oproj_steps(blk, ops_pool, osb_pool, tail=False):
                    b, qh = blk // 2, blk % 2
                    chi, clo = ctx_hi[blk % 2], ctx_lo[blk % 2]
                    # tail: deepen pipelining by alternating PSUM pools
                    # (scores banks are idle then)
                    steps = []
                    for tt in range(8):
                        def mk_tile(tt=tt):
                            osb = osb_pool.tile([128, H], BF16, tag="osb",
                                                name=f"osb_{blk}_{tt}")
                            nc.vector.memset(osb[0:1, 0:1], 0.0)
                            return osb
                        holder = {}
                        use_sps = tail and tt % 2 == 0
                        ec_per = 4 if use_sps else 2
                        for ec in range(8):
                            def mm(tt=tt, ec=ec, holder=holder, mk_tile=mk_tile,
                                   use_sps=use_sps, ec_per=ec_per):
                                if "osb" not in holder:
                                    holder["osb"] = mk_tile()
                                if ec % ec_per == 0:
                                    if use_sps:
                                        holder["op"] = sps.tile(
                                            [128, 1024], F32, tag="scores",
                                            name=f"opst_{blk}_{tt}_{ec}")
                                    else:
                                        holder["op"] = ops_pool.tile(
                                            [128, 512], F32, tag="ops",
                                            name=f"ops_{blk}_{tt}_{ec}")
                                op = holder["op"]
                                oc = (ec % ec_per) * 256
                                first = True
                                for ct, wt in ((chi, wo_hi), (chi, wo_lo), (clo, wo_hi)):
                                    nc.tensor.matmul(
                                        op[:, oc:oc + 256],
                                        ct[:, :, tt * 128:(tt + 1) * 128],
                                        wt[:, :, ec * 256:(ec + 1) * 256],
                                        start=first, stop=(ct is clo),
                                        perf_mode=DR,
                                    )
                                    first = False
                                if ec % 2 == 1:
                                    osb = holder["osb"]
                                    di = tt * 4 + ec // 2
                                    if tail and di % 2 == 1:
                                        nc.scalar.activation(
                                            out=osb[:, (ec - 1) * 256:(ec + 1) * 256],
                                            in_=op[:, (ec % ec_per - 1) * 256:
                                                   (ec % ec_per + 1) * 256],
                                            func=AF.Copy, scale=1.0 / 1024.0)
                                    else:
                                        nc.vector.tensor_scalar(
                                            out=osb[:, (ec - 1) * 256:(ec + 1) * 256],
                                            in0=op[:, (ec % ec_per - 1) * 256:
                                                   (ec % ec_per + 1) * 256],
                                            scalar1=1.0 / 1024.0, scalar2=None,
                                            op0=ALU.mult)
                                if ec == 7:
                                    nc.sync.dma_start(
                                        out=out[b, qh * 1024 + tt * 128:
                                                qh * 1024 + (tt + 1) * 128, :],
                                        in_=holder["osb"])
                            steps.append(mm)
                    return steps

                # ---------------- attention block --------------------------
                def attention_block(blk, inject, inj_start=0):
                    """inject: list of closures to interleave into the t-loop."""
                    b, qh = blk // 2, blk % 2
                    q0 = qh * 1024
                    chi, clo = ctx_hi[blk % 2], ctx_lo[blk % 2]
                    nc.vector.memset(chi[0:1, 0, 0:4].bitcast(F32), 0.0)
                    nc.vector.memset(clo[0:1, 0, 0:4].bitcast(F32), 0.0)
                    ninj = len(inject)
                    inj_i = 0
                    slots = 4 * 16  # g x t iterations
                    # software pipeline: ctx matmuls run one t behind scores,
                    # so PE never waits on the exp of the current tile.
                    pending = None  # (ctxs, ex, t)

                    def do_ctx(ctxs, ex, t):
                        if t == 0:
                            for c in ctxs:
                                nc.tensor.matmul(
                                    c[0:1, 0:1], w_hi[:, 0, 0:4].bitcast(F32),
                                    w_hi[:, 0, 0:4].bitcast(F32), start=True,
                                    stop=True, skip_group_check=True)
                        for qc2 in range(2):
                            nc.tensor.matmul(
                                ctxs[qc2],
                                vones[:, b * 16 + t, :],
                                ex[:, qc2 * 512:(qc2 + 1) * 512],
                                start=(t == 0), stop=(t == 15),
                            )

                    def drain_g(g, ctxs):
                        qp = (g % 2) * 64
                        for qc2 in range(2):
                            # fast PSUM release: copy to SBUF staging first
                            st = dsb.tile([65, 512], F32, tag="cstage",
                                          name=f"st_{blk}_{g}_{qc2}")
                            nc.vector.tensor_copy(out=st, in_=ctxs[qc2])
                            rc = dsb.tile([1, 512], F32, tag="recip",
                                          name=f"rc_{blk}_{g}_{qc2}")
                            nc.vector.reciprocal(out=rc, in_=st[64:65, :])
                            bc = dsb.tile([64, 512], F32, tag="bcast",
                                          name=f"bc_{blk}_{g}_{qc2}")
                            nc.gpsimd.partition_broadcast(bc, rc)
                            mc = dsb.tile([128, 512], F32, tag="cm",
                                          name=f"mc_{blk}_{g}_{qc2}")
                            mch = mc[qp:qp + 64, :]
                            nc.vector.tensor_mul(out=mch, in0=st[0:64, :], in1=bc)
                            dst = np.s_[qp:qp + 64, g // 2, qc2 * 512:(qc2 + 1) * 512]
                            nc.gpsimd.tensor_copy(out=chi[dst], in_=mch)
                            nc.gpsimd.tensor_sub(out=clo[dst], in0=mch, in1=chi[dst])

                    for g in range(4):
                        ctx0 = xps.tile([65, 512], F32, tag="ctx",
                                        name=f"ctx0_{blk}_{g}")
                        ctx1 = xps.tile([65, 512], F32, tag="ctx",
                                        name=f"ctx1_{blk}_{g}")
                        ctxs = (ctx0, ctx1)
                        for t in range(16):
                            sc = sps.tile([128, 1024], F32, tag="scores",
                                          name=f"sc_{blk}_{g}_{t}")
                            tok = b * S + t * 128
                            for qc in range(4):
                                nc.tensor.matmul(
                                    sc[:, qc * 256:(qc + 1) * 256],
                                    kstack[:, :, tok:tok + 128],
                                    qrep[b][g][:, :, q0 + qc * 256:q0 + (qc + 1) * 256],
                                    start=True, stop=True, perf_mode=DR,
                                )
                            ex = asb.tile([128, 1024], BF16, tag="expT",
                                          name=f"ex_{blk}_{g}_{t}")
                            nc.scalar.activation(
                                out=ex, in_=sc, func=AF.Exp,
                                bias=mask_sb[:, b, t:t + 1], scale=SCALE,
                            )
                            if pending is not None:
                                pctxs, pex, pt = pending
                                do_ctx(pctxs, pex, pt)
                                if pt == 15:
                                    drain_g(g - 1, pctxs)
                            pending = (ctxs, ex, t)
                            # interleave foreign PE work (avoid g boundaries)
                            it = g * 16 + t
                            if 1 < t < 14 and it >= inj_start:
                                eff = (it - inj_start) * 12 // 16 + 1
                                tot = (64 - inj_start) * 12 // 16
                                want = min(ninj, eff * ninj // max(tot, 1))
                                while inj_i < want:
                                    inject[inj_i]()
                                    inj_i += 1
                    pctxs, pex, pt = pending
                    do_ctx(pctxs, pex, pt)
                    drain_g(3, pctxs)
                    while inj_i < ninj:
                        inject[inj_i]()
                        inj_i += 1

                # ================= schedule ================================
                with tc.tile_pool(name="proj_sb", bufs=3) as psb, \
                     tc.tile_pool(name="proj_ps", bufs=2, space="PSUM") as pps:
                    pools["psb"], pools["pps"] = psb, pps
                    ck0_steps = qkv_chunk_steps(0)
                    ck0_steps[0]()               # chunk-0 hst DMAs first
                    nc.sync.dma_start(out=w_lo, in_=wqkvT_lo)
                    for s in ck0_steps[1:]:
                        s()
                    qkv_chunk(1)
                    rep_build(0, 0)
                    for ck in range(2, 4):       # batch-0 second half
                        qkv_chunk(ck)
                    rep_build(0, 1)
                    # batch-1 QKV injects into block 0
                    b1_steps = []
                    b1_steps.extend(qkv_chunk_steps(4))
                    b1_steps.extend(qkv_chunk_steps(5))
                    b1_steps.append(lambda: rep_build(1, 0))
                    b1_steps.extend(qkv_chunk_steps(6))
                    b1_steps.extend(qkv_chunk_steps(7))
                    b1_steps.append(lambda: rep_build(1, 1))
                    b1_steps.append(lambda: (
                        nc.sync.dma_start(out=wo_hi, in_=woT_hi),
                        nc.sync.dma_start(out=wo_lo, in_=woT_lo)))
                    attention_block(0, b1_steps, inj_start=16)

                # o_proj(k) injects into block k+1
                with tc.tile_pool(name="o_ps", bufs=2, space="PSUM") as ops_pool, \
                     tc.tile_pool(name="osb_sb", bufs=3) as osb_pool:
                    attention_block(1, oproj_steps(0, ops_pool, osb_pool))
                    attention_block(2, oproj_steps(1, ops_pool, osb_pool))
                    attention_block(3, oproj_steps(2, ops_pool, osb_pool))
                    for step in oproj_steps(3, ops_pool, osb_pool, tail=True):
                        step()
    nc.compile()
    return nc


def _hi_lo(x):
    hi = np.asarray(x, FP8NP)
    lo = np.asarray(x - hi.astype(np.float32), FP8NP)
    return hi, lo


def kernel(hidden_states, attention_mask, Wq, bq, Wk, bk, Wv, bv, Wo, bo):
    hidden_states = np.asarray(hidden_states, dtype=np.float32)
    attention_mask = np.asarray(attention_mask, dtype=np.float32)
    Wq = np.asarray(Wq, dtype=np.float32)
    Wk = np.asarray(Wk, dtype=np.float32)
    Wv = np.asarray(Wv, dtype=np.float32)
    Wo = np.asarray(Wo, dtype=np.float32)

    if "nc" not in _CACHE:
        _CACHE["nc"] = _build_program()
    nc = _CACHE["nc"]

    hsT = np.ascontiguousarray(hidden_states.reshape(TOK, H).T)   # [H, TOK]
    hsT_hi, hsT_lo = _hi_lo(hsT)
    maskp = np.ascontiguousarray(
        attention_mask.reshape(B, S // 128, 128).transpose(2, 0, 1))  # [128, B, 16]
    in_maps = []
    for c in range(N_CORES):
        w = np.concatenate([Wq[QF * c:QF * (c + 1)],
                            Wk[D * c:D * (c + 1)],
                            Wv[D * c:D * (c + 1)]], axis=0) * 32.0   # [384, H]
        # pretile to SBUF layout [128 p, 16 htile, 384 f]: h = t*128 + p
        wqkvT = np.ascontiguousarray(
            w.T.reshape(16, 128, 384).transpose(1, 0, 2))
        wq_hi, wq_lo = _hi_lo(wqkvT)
        # pretile Wo^T slice to [128 p, 2 j, H]: ctx feat = j*128 + p
        woT = np.ascontiguousarray(
            (Wo[:, QF * c:QF * (c + 1)].T * 32.0).reshape(2, 128, H)
            .transpose(1, 0, 2))
        wo_hi, wo_lo = _hi_lo(woT)
        bqkv = np.ascontiguousarray(
            np.concatenate([bq[QF * c:QF * (c + 1)], bk[D * c:D * (c + 1)],
                            bv[D * c:D * (c + 1)]]).astype(np.float32)
            .reshape(3, 128).T) * 32.0                               # [128, 3]
        in_maps.append({
            "hsT_hi": hsT_hi, "hsT_lo": hsT_lo,
            "wqkvT_hi": wq_hi, "wqkvT_lo": wq_lo,
            "woT_hi": wo_hi, "woT_lo": wo_lo,
            "bqkv": bqkv, "maskp": maskp,
        })

    _CACHE["last_in_maps"] = in_maps
    res = bass_utils.run_bass_kernel_spmd(nc, in_maps, core_ids=list(range(N_CORES)))
    acc = np.zeros((B, S, H), dtype=np.float32)
    for c in range(N_CORES):
        acc += res.results[c]["out"].astype(np.float32)
    acc += np.asarray(bo, dtype=np.float32)[None, None, :]
    return acc
